# revision 1
# baseline (speedup 1.0000x reference)
"""GEAR quantized-KV Llama attention decode step on 8 trn2 NeuronCores.

Sharding: tensor-parallel over heads (4 heads/core x 8 cores), all batches on
every core; each core computes a partial wo-product, summed on host.
"""
import os
import sys
import math

sys.path.insert(0, "/opt/trn_rl_repo")
import numpy as np
from contextlib import ExitStack

import concourse.bass as bass
import concourse.mybir as mybir
import concourse.tile as tile
from concourse import bacc, bass_isa
from concourse.bass_utils import run_bass_kernel_spmd
from concourse.masks import make_identity

B, H, D, HID = 4, 32, 128, 4096
SQ, SF, QL = 4096, 63, 1
GS, RANK = 64, 4
THETA = 10000.0
NCORES = 8
HPC = H // NCORES          # heads per core = 4
NCH = SQ // 128            # 32 s-chunks
G = SQ // GS               # 64 groups along seq (K side)
FD = D // GS               # 2 groups along head_dim (V side)
SFP = SF + 1               # 64 full-precision keys incl the new token
DT = mybir.dt
ISQD = 1.0 / math.sqrt(D)

_CACHE = {}


def _build():
    nc = bacc.Bacc("TRN2", target_bir_lowering=False)
    f32, bf16, i32 = DT.float32, DT.bfloat16, DT.int32

    hidT = nc.declare_dram_parameter("hidT", [HID, B], f32, isOutput=False)
    cost = nc.declare_dram_parameter("cost", [B, HPC * D], f32, isOutput=False)
    sint = nc.declare_dram_parameter("sint", [B, HPC * D], f32, isOutput=False)
    wT = {w: nc.declare_dram_parameter(w, [HID, HPC * D], f32, isOutput=False) for w in ("wqT", "wkT", "wvT")}
    woT = nc.declare_dram_parameter("woT", [HPC * D, HID], f32, isOutput=False)
    kcode = nc.declare_dram_parameter("kcode", [B, HPC, D, SQ], i32, isOutput=False)
    kscale = nc.declare_dram_parameter("kscale", [B, HPC, D, G], f32, isOutput=False)
    kmn = nc.declare_dram_parameter("kmn", [B, HPC, D, G], f32, isOutput=False)
    kfT = nc.declare_dram_parameter("kfT", [B, HPC, D, SF], f32, isOutput=False)
    kp = nc.declare_dram_parameter("kp", [B, HPC, 128, NCH, RANK], f32, isOutput=False)
    keyq = nc.declare_dram_parameter("keyq", [B, HPC, D, RANK], f32, isOutput=False)
    vcode = nc.declare_dram_parameter("vcode", [B, HPC, SQ, D], i32, isOutput=False)
    vscT = nc.declare_dram_parameter("vscT", [B, HPC, 128, NCH, FD], f32, isOutput=False)
    vmnT = nc.declare_dram_parameter("vmnT", [B, HPC, 128, NCH, FD], f32, isOutput=False)
    vqT = nc.declare_dram_parameter("vqT", [B, HPC, 128, NCH, RANK], f32, isOutput=False)
    vpT = nc.declare_dram_parameter("vpT", [B, HPC, 7, D], f32, isOutput=False)  # rows 0-2 zero
    vfr = nc.declare_dram_parameter("vfr", [B, HPC, SF, D], f32, isOutput=False)
    out = nc.declare_dram_parameter("out", [B, HID], f32, isOutput=True)

    AO = mybir.AluOpType
    AF = mybir.ActivationFunctionType

    with tile.TileContext(nc) as tc, ExitStack() as ctx:
        const = ctx.enter_context(tc.tile_pool(name="const", bufs=1))
        pw = ctx.enter_context(tc.tile_pool(name="pw", bufs=2))
        psC = ctx.enter_context(tc.tile_pool(name="psC", bufs=2, space="PSUM"))
        psW = ctx.enter_context(tc.tile_pool(name="psW", bufs=1, space="PSUM"))
        ictx = ctx.enter_context(ExitStack())
        psml = ictx.enter_context(tc.tile_pool(name="psml", bufs=3))
        pkc = ictx.enter_context(tc.tile_pool(name="pkc", bufs=2))
        pvt = ictx.enter_context(tc.tile_pool(name="pvt", bufs=2))
        psA = ictx.enter_context(tc.tile_pool(name="psA", bufs=2, space="PSUM"))
        psB = ictx.enter_context(tc.tile_pool(name="psB", bufs=2, space="PSUM"))

        # ---- constants ----
        id4 = const.tile([4, 4], f32)
        make_identity(nc, id4[:])
        id16 = const.tile([16, 16], f32)
        make_identity(nc, id16[:], nomemset=False)
        hid_sb = const.tile([128, HID // 128, B], f32)
        nc.sync.dma_start(out=hid_sb[:], in_=hidT[:].rearrange("(c p) b -> p c b", p=128))
        cos_sb = const.tile([B, HPC * D], f32)
        nc.sync.dma_start(out=cos_sb[:], in_=cost[:])
        sin_sb = const.tile([B, HPC * D], f32)
        nc.sync.dma_start(out=sin_sb[:], in_=sint[:])

        # ---- projections: psum[b, 512] = sum_c hidT_c^T @ wT_c ----
        proj = {}
        for wname in ("wqT", "wkT", "wvT"):
            pps = psC.tile([B, HPC * D], f32, tag="misc")
            for blk in range(4):
                slab = pw.tile([128, 8, HPC * D], f32, tag="wslab")
                nc.sync.dma_start(
                    out=slab[:],
                    in_=wT[wname][:].rearrange("(c p) n -> p c n", p=128)[:, 8 * blk:8 * blk + 8, :],
                )
                for j in range(8):
                    c = 8 * blk + j
                    nc.tensor.matmul(pps[:], hid_sb[:, c, :], slab[:, j, :],
                                     start=(c == 0), stop=(c == 31))
            sb = const.tile([B, HPC * D], f32, tag=wname)
            nc.scalar.copy(sb[:], pps[:])
            proj[wname] = sb
        q_sb, k_sb, v_sb = proj["wqT"], proj["wkT"], proj["wvT"]

        # ---- RoPE on q and k (rows [B, HPC*D]) ----
        def rope(x_sb, tagp):
            rot = const.tile([B, HPC * D], f32, tag=tagp + "rot")
            xv = x_sb[:].rearrange("b (h two d) -> b h two d", two=2, d=64)
            rv = rot[:].rearrange("b (h two d) -> b h two d", two=2, d=64)
            nc.vector.tensor_scalar(rv[:, :, 0, :], xv[:, :, 1, :], -1.0, None, AO.mult)
            nc.vector.tensor_copy(rv[:, :, 1, :], xv[:, :, 0, :])
            nc.vector.tensor_tensor(rot[:], rot[:], sin_sb[:], AO.mult)
            ro = const.tile([B, HPC * D], f32, tag=tagp + "ro")
            nc.vector.tensor_tensor(ro[:], x_sb[:], cos_sb[:], AO.mult)
            nc.vector.tensor_tensor(ro[:], ro[:], rot[:], AO.add)
            return ro
        qro = rope(q_sb, "q")
        kro = rope(k_sb, "k")

        # per-head transposed columns: qscT [128, h, b] (scaled by 1/sqrt(D)), kT
        qscT = const.tile([128, HPC, B], f32)
        kT = const.tile([128, HPC, B], f32)
        for h in range(HPC):
            pq = psC.tile([128, B], f32, tag="misc")
            nc.tensor.transpose(pq[:], qro[0:B, h * D:(h + 1) * D], id4[:])
            nc.scalar.mul(qscT[:, h, :], pq[:], ISQD)
            pk = psC.tile([128, B], f32, tag="misc")
            nc.tensor.transpose(pk[:], kro[0:B, h * D:(h + 1) * D], id4[:])
            nc.scalar.copy(kT[:, h, :], pk[:])

        rows_sb = const.tile([16, 128], f32)
        woin_ps = psW.tile([128, 16], f32)

        # ---- per (b, h) attention ----
        for b in range(B):
            for h in range(HPC):
                idx = h * B + b
                qcol = qscT[:, h, b:b + 1]

                kc_bf = pkc.tile([128, SQ], bf16, tag="kc")
                nc.gpsimd.dma_start(out=kc_bf[:], in_=kcode[b, h])
                ksc = psml.tile([128, G], f32, tag="ksc")
                nc.sync.dma_start(out=ksc[:], in_=kscale[b, h])
                kmn_sb = psml.tile([128, G], f32, tag="kmn")
                nc.sync.dma_start(out=kmn_sb[:], in_=kmn[b, h])
                kfp = psml.tile([128, SFP], f32, tag="kfp")
                nc.sync.dma_start(out=kfp[:, 0:SF], in_=kfT[b, h])
                kp_sb = psml.tile([128, NCH, RANK], f32, tag="kp")
                nc.sync.dma_start(out=kp_sb[:], in_=kp[b, h])
                keyq_sb = psml.tile([128, RANK], f32, tag="keyq")
                nc.sync.dma_start(out=keyq_sb[:], in_=keyq[b, h])
                vt = pvt.tile([128, NCH, 131], bf16, tag="vt")
                nc.gpsimd.dma_start(out=vt[:, :, 0:128],
                                    in_=vcode[b, h].rearrange("(c p) d -> p c d", p=128))
                nc.gpsimd.dma_start(out=vt[:, :, 128:130], in_=vmnT[b, h])
                aw3 = psml.tile([128, NCH, 7], bf16, tag="aw3")
                nc.gpsimd.dma_start(out=aw3[:, :, 3:7], in_=vqT[b, h])
                vsc = psml.tile([128, NCH, FD], f32, tag="vsc")
                nc.sync.dma_start(out=vsc[:], in_=vscT[b, h])
                vpT_sb = psml.tile([7, D], f32, tag="vpT")
                nc.sync.dma_start(out=vpT_sb[:], in_=vpT[b, h])
                vf_sb = psml.tile([SFP, D], f32, tag="vf")
                nc.sync.dma_start(out=vf_sb[0:SF, :], in_=vfr[b, h])
                # new-token k/v into the full-precision blocks
                nc.vector.tensor_copy(kfp[:, SF:SFP], kT[:, h, b:b + 1])
                nc.sync.dma_start(out=vf_sb[SF:SFP, :], in_=v_sb[b:b + 1, h * D:(h + 1) * D])

                # quant K scores: psk[s, 2c + g'] over chunks
                qs = psml.tile([128, G], bf16, tag="qs")
                nc.vector.tensor_scalar(qs[:], ksc[:], qcol, None, AO.mult)
                psk = psA.tile([128, 2 * NCH], f32, tag="psk")
                for c in range(NCH):
                    nc.tensor.matmul(psk[:, 2 * c:2 * c + 2], kc_bf[:, c * 128:(c + 1) * 128],
                                     qs[:, 2 * c:2 * c + 2], start=True, stop=True)
                # misc: kf scores [0:64, 0:1]; qr row [0:1, 32:36]; mn bias row [0:1, 64:128]
                psm = psC.tile([128, 128], f32, tag="misc")
                nc.tensor.matmul(psm[0:SFP, 0:1], kfp[:], qcol, start=True, stop=True)
                nc.tensor.matmul(psm[0:1, 32:36], qcol, keyq_sb[:], start=True, stop=True)
                nc.tensor.matmul(psm[0:1, 64:128], qcol, kmn_sb[:], start=True, stop=True)

                qr_sb = psml.tile([1, RANK], f32, tag="qr")
                nc.scalar.copy(qr_sb[:], psm[0:1, 32:36])
                qrb = psml.tile([128, RANK], f32, tag="qrb")
                nc.gpsimd.partition_broadcast(qrb[:], qr_sb[:])
                bias_r = psml.tile([1, G], f32, tag="biasr")
                nc.scalar.copy(bias_r[:], psm[0:1, 64:128])
                bias_bc = psml.tile([128, G], f32, tag="biasbc")
                nc.gpsimd.partition_broadcast(bias_bc[:], bias_r[:])

                lrt = psml.tile([128, NCH, RANK], f32, tag="lrt")
                nc.vector.tensor_tensor(lrt[:], kp_sb[:],
                                        qrb[:, None, :].to_broadcast((128, NCH, RANK)), AO.mult)
                lr = psml.tile([128, NCH], f32, tag="lr")
                nc.vector.reduce_sum(lr[:], lrt[:], axis=mybir.AxisListType.X)

                att = psml.tile([128, NCH + 1], f32, tag="att")
                pskv = psk[:].rearrange("p (c two) -> p c two", two=2)
                bbv = bias_bc[:].rearrange("p (c two) -> p c two", two=2)
                nc.vector.tensor_tensor(att[0:64, 0:NCH], pskv[0:64, :, 0], lr[0:64, :], AO.add)
                nc.vector.tensor_tensor(att[0:64, 0:NCH], att[0:64, 0:NCH], bbv[0:64, :, 0], AO.add)
                nc.vector.tensor_tensor(att[64:128, 0:NCH], pskv[64:128, :, 1], lr[64:128, :], AO.add)
                nc.vector.tensor_tensor(att[64:128, 0:NCH], att[64:128, 0:NCH], bbv[64:128, :, 1], AO.add)
                nc.vector.memset(att[:, NCH:NCH + 1], -1e9)
                nc.vector.tensor_copy(att[0:SFP, NCH:NCH + 1], psm[0:SFP, 0:1])

                # softmax over all 128 x 33 entries
                m1 = psml.tile([128, 1], f32, tag="m1")
                nc.vector.reduce_max(m1[:], att[:], axis=mybir.AxisListType.X)
                mg = psml.tile([128, 1], f32, tag="mg")
                nc.gpsimd.partition_all_reduce(mg[:], m1[:], 128, bass_isa.ReduceOp.max)
                negm = psml.tile([128, 1], f32, tag="negm")
                nc.vector.tensor_scalar(negm[:], mg[:], -1.0, None, AO.mult)
                e = psml.tile([128, NCH + 1], bf16, tag="e")
                ssum = psml.tile([128, 1], f32, tag="ssum")
                nc.scalar.activation(e[:], att[:], AF.Exp, bias=negm[:, 0:1], scale=1.0,
                                     alpha=0.0, accum_out=ssum[:])
                sg = psml.tile([128, 1], f32, tag="sg")
                nc.gpsimd.partition_all_reduce(sg[:], ssum[:], 128, bass_isa.ReduceOp.add)
                recip = psml.tile([128, 1], f32, tag="recip")
                nc.vector.reciprocal(recip[:], sg[:])

                # build lhsT cols: 0 = aw, 1-2 = aw*vs, (3-6 = vq already)
                ev = e[:, 0:NCH, None]
                nc.vector.tensor_scalar(aw3[:, :, 0:1], ev, recip[:, 0:1], None, AO.mult)
                nc.vector.scalar_tensor_tensor(aw3[:, :, 1:3], ev.to_broadcast((128, NCH, FD)),
                                               recip[:, 0:1], vsc[:], AO.mult, AO.mult)
                nc.vector.tensor_scalar(vt[:, :, 130:131], ev, recip[:, 0:1], None, AO.mult)
                awf = psml.tile([SFP, 1], f32, tag="awf")
                nc.vector.tensor_scalar(awf[:], e[0:SFP, NCH:NCH + 1], recip[0:SFP, 0:1],
                                        None, AO.mult)

                psv = psB.tile([7, 131], f32, tag="psv")
                for c in range(NCH):
                    nc.tensor.matmul(psv[:], aw3[:, c, :], vt[:, c, :],
                                     start=(c == 0), stop=(c == NCH - 1))

                # mn scalars at partition 0; broadcast to partitions 1,2
                mn2 = psml.tile([3, FD], f32, tag="mn2")
                nc.scalar.copy(mn2[0:1, :], psv[0:1, 128:130])
                mn2b = psml.tile([3, FD], f32, tag="mn2b")
                nc.gpsimd.partition_broadcast(mn2b[:], mn2[0:1, :], channels=3)
                stage = psml.tile([3, 128], f32, tag="stage")
                nc.vector.tensor_scalar(stage[0:3, 0:64], psv[0:3, 0:64], mn2b[0:3, 0:1],
                                        None, AO.add)
                nc.vector.tensor_scalar(stage[0:3, 64:128], psv[0:3, 64:128], mn2b[0:3, 1:2],
                                        None, AO.add)
                nc.sync.dma_start(out=rows_sb[idx:idx + 1, 0:64], in_=stage[1:2, 0:64])
                nc.sync.dma_start(out=rows_sb[idx:idx + 1, 64:128], in_=stage[2:3, 64:128])

                vr_sb = psml.tile([7, 1], f32, tag="vr")
                nc.scalar.copy(vr_sb[:], psv[:, 130:131])
                nc.tensor.matmul(woin_ps[:, idx:idx + 1], vpT_sb[:], vr_sb[:],
                                 start=True, stop=False)
                nc.tensor.matmul(woin_ps[:, idx:idx + 1], vf_sb[:], awf[:],
                                 start=False, stop=True)

        # ---- tail: transpose rows, combine, wo matmul ----
        ictx.close()
        psO = ctx.enter_context(tc.tile_pool(name="psO", bufs=1, space="PSUM"))
        trp = psC.tile([128, 16], f32, tag="misc")
        nc.tensor.transpose(trp[:], rows_sb[:], id16[:])
        tr_sb = const.tile([128, 16], f32)
        nc.scalar.copy(tr_sb[:], trp[:])
        woin_sb = const.tile([128, 16], f32)
        nc.vector.tensor_tensor(woin_sb[:], tr_sb[:], woin_ps[:], AO.add)

        wo_sb = const.tile([128, HPC, HID], f32)
        nc.sync.dma_start(out=wo_sb[:], in_=woT[:].rearrange("(c p) n -> p c n", p=128))
        for half in range(2):
            po = psO.tile([B, HID // 2], f32, tag="po")
            for h in range(HPC):
                for nb in range(4):
                    j0 = half * 2048 + nb * 512
                    nc.tensor.matmul(po[:, nb * 512:(nb + 1) * 512],
                                     woin_sb[:, h * B:(h + 1) * B], wo_sb[:, h, j0:j0 + 512],
                                     start=(h == 0), stop=(h == HPC - 1))
            osb = const.tile([B, HID // 2], f32, tag=f"osb{half}")
            nc.scalar.copy(osb[:], po[:])
            nc.sync.dma_start(out=out[:, half * 2048:(half + 1) * 2048], in_=osb[:])

    nc.compile()
    return nc


def _host_prep(inputs):
    hs = np.asarray(inputs["hidden_states"], np.float32)
    pos = np.asarray(inputs["position_ids"])
    inv = 1.0 / (THETA ** (np.arange(0, D, 2, dtype=np.float32) / D))
    fr = pos[:, 0].astype(np.float32)[:, None] * inv[None, :]
    emb = np.concatenate([fr, fr], axis=1)
    cos_b = np.cos(emb).astype(np.float32)
    sin_b = np.sin(emb).astype(np.float32)
    cost = np.ascontiguousarray(np.tile(cos_b, (1, HPC)))
    sint = np.ascontiguousarray(np.tile(sin_b, (1, HPC)))
    hidT = np.ascontiguousarray(hs[:, 0, :].T)

    wq, wk, wv, wo = (np.asarray(inputs[k], np.float32) for k in ("wq", "wk", "wv", "wo"))
    in_maps = []
    for core in range(NCORES):
        h0 = core * HPC
        sl = slice(h0 * D, (h0 + HPC) * D)
        hsl = slice(h0, h0 + HPC)

        def rearr(x):  # [B,HPC,SQ,w] -> [B,HPC,128,NCH,w]
            w = x.shape[-1]
            return np.ascontiguousarray(
                x.reshape(B, HPC, NCH, 128, w).transpose(0, 1, 3, 2, 4))

        vp = np.asarray(inputs["value_p"], np.float32)[:, hsl]  # [B,HPC,D,R]
        vpT = np.zeros((B, HPC, 7, D), np.float32)
        vpT[:, :, 3:7, :] = vp.transpose(0, 1, 3, 2)
        m = {
            "hidT": hidT, "cost": cost, "sint": sint,
            "wqT": np.ascontiguousarray(wq[sl].T),
            "wkT": np.ascontiguousarray(wk[sl].T),
            "wvT": np.ascontiguousarray(wv[sl].T),
            "woT": np.ascontiguousarray(wo[:, sl].T),
            "kcode": np.ascontiguousarray(np.asarray(inputs["k_quant"], np.int32)[:, hsl]),
            "kscale": np.ascontiguousarray(np.asarray(inputs["k_scale"], np.float32)[:, hsl]),
            "kmn": np.ascontiguousarray(np.asarray(inputs["k_mn"], np.float32)[:, hsl]),
            "kfT": np.ascontiguousarray(
                np.asarray(inputs["k_full"], np.float32)[:, hsl].transpose(0, 1, 3, 2)),
            "kp": rearr(np.asarray(inputs["key_p"], np.float32)[:, hsl]),
            "keyq": np.ascontiguousarray(np.asarray(inputs["key_q"], np.float32)[:, hsl]),
            "vcode": np.ascontiguousarray(np.asarray(inputs["v_quant"], np.int32)[:, hsl]),
            "vscT": rearr(np.asarray(inputs["v_scale"], np.float32)[:, hsl]),
            "vmnT": rearr(np.asarray(inputs["v_mn"], np.float32)[:, hsl]),
            "vqT": rearr(np.asarray(inputs["value_q"], np.float32)[:, hsl]),
            "vpT": vpT,
            "vfr": np.ascontiguousarray(np.asarray(inputs["v_full"], np.float32)[:, hsl]),
        }
        in_maps.append(m)
    return in_maps


def kernel(**inputs):
    if "nc" not in _CACHE:
        _CACHE["nc"] = _build()
    nc = _CACHE["nc"]
    in_maps = _host_prep(inputs)
    res = run_bass_kernel_spmd(nc, in_maps, list(range(NCORES)),
                               trace=bool(os.environ.get("K_TRACE")))
    kernel.last = res
    total = np.zeros((B, HID), np.float32)
    for r in res.results:
        total += r["out"]
    return total.reshape(B, QL, HID)



# revision 2
# speedup vs baseline: 1.0041x; 1.0041x over previous
"""GEAR quantized-KV Llama attention decode step on 8 trn2 NeuronCores.

Sharding: tensor-parallel over heads (4 heads/core x 8 cores), all batches on
every core; each core computes a partial wo-product, summed on host.

v2: fp8 codes (exact for 0..15), bf16 weights, host-side layout packing so all
DMAs are large+contiguous, DoubleRow fp8 V-side matmuls, batched softmax.
"""
import os
import sys
import math

sys.path.insert(0, "/opt/trn_rl_repo")
import numpy as np
import ml_dtypes
from contextlib import ExitStack

import concourse.bass as bass
import concourse.mybir as mybir
import concourse.tile as tile
from concourse import bacc, bass_isa
from concourse.bass_utils import run_bass_kernel_spmd
from concourse.masks import make_identity

B, H, D, HID = 4, 32, 128, 4096
SQ, SF, QL = 4096, 63, 1
GS, RANK = 64, 4
THETA = 10000.0
NCORES = 8
HPC = H // NCORES          # heads per core = 4
U = HPC * B                # 16 (h, b) pairs per core
NCH = SQ // 128            # 32 s-chunks
G = SQ // GS               # 64 groups along seq (K side)
FD = D // GS               # 2 groups along head_dim (V side)
SFP = SF + 1               # 64 full-precision keys incl the new token
VT_W = 144                 # vt cols: 128 codes + 2 mn + 1 aw + pad (16B-mult stride)
AW_W = 16                  # aw stationary cols: aw, aw*vs0, aw*vs1, vq0-3, pad
DT = mybir.dt
ISQD = 1.0 / math.sqrt(D)
QS_SC = 64.0               # qs pre-scale (fp8 range)
AW_SC = float(2 ** 12)     # aw pre-scale
VS_SC = float(2 ** 8)      # extra vs pre-scale (total aw*vs = 2^20)
MN_SC = 16.0               # vmn/vq pre-scale
F8 = ml_dtypes.float8_e4m3

_CACHE = {}


def _build():
    nc = bacc.Bacc("TRN2", target_bir_lowering=False)
    f32, bf16, i32, f8 = DT.float32, DT.bfloat16, DT.int32, DT.float8e4

    hidb = nc.declare_dram_parameter("hidb", [128, HID // 256, 2, 16], f8, isOutput=False)
    cost = nc.declare_dram_parameter("cost", [B, HPC * D], f32, isOutput=False)
    sint = nc.declare_dram_parameter("sint", [B, HPC * D], f32, isOutput=False)
    wT = {w: nc.declare_dram_parameter(w, [128, HID // 256, 2, HPC * D], f8, isOutput=False)
          for w in ("wqT", "wkT", "wvT")}
    woT = nc.declare_dram_parameter("woT", [128, HPC * D // 128, HID], bf16, isOutput=False)
    kcode = nc.declare_dram_parameter("kcode", [U, D, SQ], f8, isOutput=False)
    ksc64 = nc.declare_dram_parameter("ksc64", [128, U, G], bf16, isOutput=False)
    kmn = nc.declare_dram_parameter("kmn", [128, U, G], bf16, isOutput=False)
    kfp = nc.declare_dram_parameter("kfp", [128, U, 128], bf16, isOutput=False)
    kp = nc.declare_dram_parameter("kp", [128, U, NCH, RANK], bf16, isOutput=False)
    keyq = nc.declare_dram_parameter("keyq", [128, U, RANK], bf16, isOutput=False)
    vtall = nc.declare_dram_parameter("vtall", [U, 128, NCH * VT_W], f8, isOutput=False)
    vqst = nc.declare_dram_parameter("vqst", [128, U, NCH * AW_W], f8, isOutput=False)
    vsp = nc.declare_dram_parameter("vsp", [128, U, NCH, FD], bf16, isOutput=False)
    vpT = nc.declare_dram_parameter("vpT", [7, U, D], bf16, isOutput=False)  # rows 0-2 zero
    vfr = nc.declare_dram_parameter("vfr", [SFP, U, D], bf16, isOutput=False)  # row 63 zero
    out = nc.declare_dram_parameter("out", [B, HID], f32, isOutput=True)

    AO = mybir.AluOpType
    AF = mybir.ActivationFunctionType
    PM = mybir.MatmulPerfMode

    with tile.TileContext(nc) as tc, ExitStack() as ctx:
        const = ctx.enter_context(tc.tile_pool(name="const", bufs=1))
        pw = ctx.enter_context(tc.tile_pool(name="pw", bufs=2))
        psC = ctx.enter_context(tc.tile_pool(name="psC", bufs=1, space="PSUM"))
        psW = ctx.enter_context(tc.tile_pool(name="psW", bufs=1, space="PSUM"))
        ictx = ctx.enter_context(ExitStack())
        pkc = ictx.enter_context(tc.tile_pool(name="pkc", bufs=6))
        psm_pool = ictx.enter_context(tc.tile_pool(name="psmp", bufs=2, space="PSUM"))
        psK = ictx.enter_context(tc.tile_pool(name="psK", bufs=1, space="PSUM"))
        psV = ictx.enter_context(tc.tile_pool(name="psV", bufs=2, space="PSUM"))

        # ---- persistent SBUF tiles ----
        id4 = const.tile([4, 4], f32)
        make_identity(nc, id4[:])
        id16 = const.tile([16, 16], bf16)
        make_identity(nc, id16[:], nomemset=False)
        hid_sb = const.tile([128, HID // 256, 2, 16], f8)
        nc.sync.dma_start(out=hid_sb[:], in_=hidb[:])
        cos_sb = const.tile([B, HPC * D], f32)
        nc.sync.dma_start(out=cos_sb[:], in_=cost[:])
        sin_sb = const.tile([B, HPC * D], f32)
        nc.sync.dma_start(out=sin_sb[:], in_=sint[:])

        # big attention inputs (per-pair slices of mega tiles).
        # Mega input tiles: allocated here, DMA'd at their consumption point so
        # bytes flow through the queues in consumption order.
        ksc_sb = const.tile([128, U, G], bf16)
        kmn_sb = const.tile([128, U, G], bf16)
        kfp_sb = const.tile([128, U, 128], bf16)
        kp_sb = const.tile([128, U, NCH, RANK], bf16)
        keyq_sb = const.tile([128, U, RANK], bf16)
        vt_sb = const.tile([128, U, NCH, VT_W], f8)
        aw_st = const.tile([128, U, NCH, AW_W], f8)
        vsp_sb = const.tile([128, U, NCH, FD], bf16)
        vpT_sb = const.tile([7, U, D], bf16)
        vf_sb = const.tile([SFP, U, D], bf16)

        # ---- projections (fp8 DoubleRow, 256-deep contraction chunks) ----
        proj = {}
        for wname in ("wqT", "wkT", "wvT"):
            pps = psC.tile([B, HPC * D], f32, tag="misc")
            for blk in range(4):
                slab = pw.tile([128, 4, 2, HPC * D], f8, tag="wslab")
                nc.sync.dma_start(out=slab[:], in_=wT[wname][:, 4 * blk:4 * blk + 4, :, :])
                for j in range(4):
                    c = 4 * blk + j
                    nc.tensor.matmul(pps[:], hid_sb[:, c, :, 0:B], slab[:, j, :, :],
                                     start=(c == 0), stop=(c == 15),
                                     perf_mode=PM.DoubleRow)
            sb = const.tile([B, HPC * D], f32, tag=wname)
            nc.scalar.mul(sb[:], pps[:], 1.0 / 32.0)
            proj[wname] = sb
        q_sb, k_sb, v_sb = proj["wqT"], proj["wkT"], proj["wvT"]

        # ---- RoPE on q and k (rows [B, HPC*D]) ----
        def rope(x_sb, tagp):
            rot = const.tile([B, HPC * D], f32, tag="rot")
            xv = x_sb[:].rearrange("b (h two d) -> b h two d", two=2, d=64)
            rv = rot[:].rearrange("b (h two d) -> b h two d", two=2, d=64)
            nc.vector.tensor_scalar(rv[:, :, 0, :], xv[:, :, 1, :], -1.0, None, AO.mult)
            nc.vector.tensor_copy(rv[:, :, 1, :], xv[:, :, 0, :])
            nc.vector.tensor_tensor(rot[:], rot[:], sin_sb[:], AO.mult)
            ro = const.tile([B, HPC * D], f32, tag=tagp + "ro")
            nc.vector.tensor_tensor(ro[:], x_sb[:], cos_sb[:], AO.mult)
            nc.vector.tensor_tensor(ro[:], ro[:], rot[:], AO.add)
            return ro
        qro = rope(q_sb, "q")
        kro = rope(k_sb, "k")

        # per-pair transposed q columns: qscT [128, u] f32 (scaled by 1/sqrt(D)), kT bf16
        qscT = const.tile([128, U], f32)
        kT_all = const.tile([128, U], bf16)
        for h in range(HPC):
            pq = psC.tile([128, B], f32, tag="misc")
            nc.tensor.transpose(pq[:], qro[0:B, h * D:(h + 1) * D], id4[:])
            nc.scalar.mul(qscT[:, h * B:(h + 1) * B], pq[:], ISQD)
            pk = psC.tile([128, B], f32, tag="misc")
            nc.tensor.transpose(pk[:], kro[0:B, h * D:(h + 1) * D], id4[:])
            nc.scalar.copy(kT_all[:, h * B:(h + 1) * B], pk[:])
        qb_all = const.tile([128, U], bf16)
        nc.scalar.copy(qb_all[:], qscT[:])
        # K-side mega DMAs (issued after the weight slabs on the sync queue)
        nc.sync.dma_start(out=ksc_sb[:], in_=ksc64[:])
        nc.sync.dma_start(out=kmn_sb[:], in_=kmn[:])
        nc.sync.dma_start(out=kfp_sb[:], in_=kfp[:])
        nc.sync.dma_start(out=kp_sb[:], in_=kp[:])
        nc.sync.dma_start(out=keyq_sb[:], in_=keyq[:])
        # new-token k into full-precision key block (col 63 of kfp)
        nc.vector.tensor_copy(kfp_sb[:, :, 63:64], kT_all[:, :, None])
        v_bf = const.tile([B, HPC * D], bf16)
        nc.vector.tensor_copy(v_bf[:], v_sb[:])

        # qs for all pairs: [128, U, G] fp8 = ksc64 * qcol
        qs_all = const.tile([128, U, G], f8)
        nc.vector.tensor_tensor(qs_all[:], ksc_sb[:],
                                qb_all[:, :, None].to_broadcast((128, U, G)), AO.mult)

        # ---- K phase: per pair, quant scores + kf/qr/bias matmuls ----
        psk = psK.tile([128, U, G], f32)        # 4KB/partition, 2 banks
        kf_sb = const.tile([128, U], f32)
        qrbias_sb = const.tile([1, U, 96], f32)
        kc_tiles = []
        for u in range(U):
            kc = pkc.tile([128, SQ], f8, tag="kc", name=f"kc{u}")
            nc.scalar.dma_start(out=kc[:], in_=kcode[u])
            kc_tiles.append(kc)
        # V-side bytes strictly after kc in the scalar queue's issue order
        for u in range(U):
            nc.scalar.dma_start(out=vt_sb[:, u],
                                in_=vtall[u].rearrange("p (c w) -> p c w", w=VT_W))
        nc.scalar.dma_start(out=aw_st[:], in_=vqst[:].rearrange("p u (c w) -> p u c w", w=AW_W))
        nc.scalar.dma_start(out=vsp_sb[:], in_=vsp[:])
        nc.scalar.dma_start(out=vpT_sb[:], in_=vpT[:])
        nc.scalar.dma_start(out=vf_sb[:], in_=vfr[:])
        for u in range(U):
            h, b = divmod(u, B)
            nc.scalar.dma_start(out=vf_sb[SF:SFP, u, :],
                                in_=v_bf[b:b + 1, h * D:(h + 1) * D])
        for u in range(U):
            kc = kc_tiles[u]
            for c in range(NCH):
                nc.tensor.matmul(psk[:, u, 2 * c:2 * c + 2], kc[:, c * 128:(c + 1) * 128],
                                 qs_all[:, u, 2 * c:2 * c + 2], start=True, stop=True)
            psm = psm_pool.tile([128, 128], f32, tag="psm")
            nc.tensor.matmul(psm[:, 0:1], kfp_sb[:, u, :], qb_all[:, u:u + 1],
                             start=True, stop=True)
            nc.tensor.matmul(psm[0:1, 32:36], qb_all[:, u:u + 1], keyq_sb[:, u, :],
                             start=True, stop=True)
            nc.tensor.matmul(psm[0:1, 64:128], qb_all[:, u:u + 1], kmn_sb[:, u, :],
                             start=True, stop=True)
            nc.vector.tensor_copy(qrbias_sb[0:1, u, :], psm[0:1, 32:128])
            nc.vector.tensor_copy(kf_sb[:, u:u + 1], psm[:, 0:1])

        # ---- batched low-rank + att assembly ----
        qrb = const.tile([128, U, RANK], f32)
        nc.gpsimd.partition_broadcast(qrb[:], qrbias_sb[0:1, :, 0:4])
        bias_bc = const.tile([128, U, G], f32)
        nc.gpsimd.partition_broadcast(bias_bc[:], qrbias_sb[0:1, :, 32:96])
        lrt = const.tile([128, U, NCH, RANK], bf16)
        nc.gpsimd.tensor_tensor(lrt[:], kp_sb[:],
                                qrb[:, :, None, :].to_broadcast((128, U, NCH, RANK)), AO.mult)
        lr = const.tile([128, U, NCH], f32)
        nc.vector.reduce_sum(lr[:], lrt[:], axis=mybir.AxisListType.X)

        att = const.tile([128, U, NCH + 1], f32)
        mask_col = const.tile([128, 1], f32)
        nc.vector.memset(mask_col[0:SFP, :], 0.0)
        nc.vector.memset(mask_col[SFP:128, :], -1e9)
        pskv = psk[:].rearrange("p u (c two) -> p u c two", two=2)
        bbv = bias_bc[:].rearrange("p u (c two) -> p u c two", two=2)
        for gp in range(2):
            sl = slice(64 * gp, 64 * gp + 64)
            nc.vector.scalar_tensor_tensor(att[sl, :, 0:NCH], pskv[sl, :, :, gp],
                                           1.0 / QS_SC, lr[sl, :, :], AO.mult, AO.add)
            nc.vector.tensor_tensor(att[sl, :, 0:NCH], att[sl, :, 0:NCH],
                                    bbv[sl, :, :, gp], AO.add)
        nc.vector.tensor_tensor(att[:, :, NCH:NCH + 1], kf_sb[:, :, None],
                                mask_col[:, None, :].to_broadcast((128, U, 1)), AO.add)

        # ---- batched softmax (no max subtraction needed: |att| < 10) ----
        e_all = const.tile([128, U, NCH + 1], bf16)
        nc.scalar.activation(e_all[:], att[:], AF.Exp, bias=0.0, scale=1.0, alpha=0.0)
        ssum = const.tile([128, U], f32)
        nc.vector.reduce_sum(ssum[:], e_all[:], axis=mybir.AxisListType.X)
        sg = const.tile([128, U], f32)
        nc.gpsimd.partition_all_reduce(sg[:], ssum[:], 128, bass_isa.ReduceOp.add)
        recip = const.tile([128, U], f32)
        nc.vector.reciprocal(recip[:], sg[:])
        recip_s = const.tile([128, U], f32)
        nc.vector.tensor_scalar(recip_s[:], recip[:], AW_SC, None, AO.mult)

        # ---- batched aw build (fp8 stationary cols + vt aw col + awf) ----
        ev = e_all[:, :, 0:NCH]
        nc.vector.tensor_tensor(aw_st[:, :, :, 0], ev,
                                recip_s[:, :, None].to_broadcast((128, U, NCH)), AO.mult)
        nc.gpsimd.tensor_tensor(vt_sb[:, :, :, 130], ev,
                                recip_s[:, :, None].to_broadcast((128, U, NCH)), AO.mult)
        nc.vector.tensor_tensor(aw_st[:, :, :, 1:3],
                                aw_st[:, :, :, 0:1].to_broadcast((128, U, NCH, FD)),
                                vsp_sb[:], AO.mult)
        awf = const.tile([SFP, U], bf16)
        nc.gpsimd.tensor_tensor(awf[:], e_all[0:SFP, :, NCH],
                                recip[0:SFP, :], AO.mult)

        # ---- V phase: per pair DoubleRow matmuls + vf/vp into woin psum ----
        rows_sb = const.tile([16, 128], bf16)
        woin_ps = psW.tile([128, 16], f32)
        psv_sb = const.tile([7, U, VT_W], bf16)
        for u in range(U):
            psv = psV.tile([7, VT_W], f32, tag="psv")
            for cp in range(NCH // 2):
                nc.tensor.matmul(psv[:, 0:131], aw_st[:, u, 2 * cp:2 * cp + 2, 0:7],
                                 vt_sb[:, u, 2 * cp:2 * cp + 2, 0:131],
                                 start=(cp == 0), stop=(cp == NCH // 2 - 1),
                                 perf_mode=PM.DoubleRow)
            nc.vector.tensor_copy(psv_sb[:, u, 0:131], psv[:, 0:131])
            nc.sync.dma_start(out=rows_sb[u:u + 1, 0:64], in_=psv_sb[1:2, u, 0:64])
            nc.sync.dma_start(out=rows_sb[u:u + 1, 64:128], in_=psv_sb[2:3, u, 64:128])
            nc.tensor.matmul(woin_ps[:, u:u + 1], vpT_sb[:, u, :], psv_sb[:, u, 130:131],
                             start=True, stop=False)
            nc.tensor.matmul(woin_ps[:, u:u + 1], vf_sb[:, u, :], awf[:, u:u + 1],
                             start=False, stop=True)

        # ---- tail: transpose rows, add mn, combine, wo matmul ----
        ictx.close()
        psO = ctx.enter_context(tc.tile_pool(name="psO", bufs=2, space="PSUM"))
        trp = psC.tile([128, 16], bf16, tag="misc")
        nc.tensor.transpose(trp[:], rows_sb[:], id16[:])
        tr_sb = const.tile([128, 16], f32)
        nc.scalar.copy(tr_sb[:], trp[:])
        # mn sums: psv_sb[0, u, 128:130] * 2^-16  -> broadcast d<64 | d>=64
        mn_row = const.tile([1, U, FD], f32)
        nc.scalar.mul(mn_row[:], psv_sb[0:1, :, 128:130], 1.0 / (AW_SC * MN_SC))
        mn_full = const.tile([128, U, FD], f32)
        nc.gpsimd.partition_broadcast(mn_full[:], mn_row[0:1, :, :])
        woin_sb = const.tile([128, 16], f32)
        nc.vector.scalar_tensor_tensor(woin_sb[0:64, :], tr_sb[0:64, :],
                                       1.0 / (AW_SC * VS_SC), mn_full[0:64, :, 0],
                                       AO.mult, AO.add)
        nc.vector.scalar_tensor_tensor(woin_sb[64:128, :], tr_sb[64:128, :],
                                       1.0 / (AW_SC * VS_SC), mn_full[64:128, :, 1],
                                       AO.mult, AO.add)
        woin_bf = const.tile([128, 16], bf16)
        nc.vector.tensor_tensor(woin_bf[:], woin_sb[:], woin_ps[:], AO.add)

        for quarter in range(4):
            po = psO.tile([B, HID // 4], f32, tag="po")
            for nb in range(2):
                j0 = quarter * 1024 + nb * 512
                woslab = pw.tile([128, HPC, 512], bf16, tag="woslab", bufs=4)
                nc.scalar.dma_start(out=woslab[:], in_=woT[:, :, j0:j0 + 512])
                for h in range(HPC):
                    nc.tensor.matmul(po[:, nb * 512:(nb + 1) * 512],
                                     woin_bf[:, h * B:(h + 1) * B], woslab[:, h, :],
                                     start=(h == 0), stop=(h == HPC - 1))
            osb = const.tile([B, HID // 4], f32, tag="osb", bufs=2)
            nc.scalar.copy(osb[:], po[:])
            nc.sync.dma_start(out=out[:, quarter * 1024:(quarter + 1) * 1024], in_=osb[:])

    nc.compile()
    return nc


def _host_prep(inputs):
    hs = np.asarray(inputs["hidden_states"], np.float32)
    pos = np.asarray(inputs["position_ids"])
    inv = 1.0 / (THETA ** (np.arange(0, D, 2, dtype=np.float32) / D))
    fr = pos[:, 0].astype(np.float32)[:, None] * inv[None, :]
    emb = np.concatenate([fr, fr], axis=1)
    cost = np.ascontiguousarray(np.tile(np.cos(emb), (1, HPC))).astype(np.float32)
    sint = np.ascontiguousarray(np.tile(np.sin(emb), (1, HPC))).astype(np.float32)
    hid8 = np.zeros((128, HID // 256, 2, 16), np.float32)
    hid8[:, :, :, 0:B] = hs[:, 0, :].T.reshape(16, 2, 128, B).transpose(2, 0, 1, 3)
    hidb = hid8.astype(F8)

    wq, wk, wv, wo = (np.asarray(inputs[k], np.float32) for k in ("wq", "wk", "wv", "wo"))

    def hb(x, u_axis=None):
        # [.., B, HPC, ..] -> pair-major [h*B+b] handled by callers
        return np.ascontiguousarray(x).astype(ml_dtypes.bfloat16)

    in_maps = []
    for core in range(NCORES):
        h0 = core * HPC
        sl = slice(h0 * D, (h0 + HPC) * D)
        hsl = slice(h0, h0 + HPC)

        def pair_major(x):
            # x: [B, HPC, ...] -> [U = h*B+b, ...]
            s = x.shape
            return np.ascontiguousarray(
                x.transpose(1, 0, *range(2, x.ndim)).reshape(U, *s[2:]))

        def wchunk(x):  # [4096, N] -> [128, 16, 2, N] fp8, pre-scaled x32
            n = x.shape[-1]
            return np.ascontiguousarray(
                (x * 32.0).reshape(16, 2, 128, n).transpose(2, 0, 1, 3)).astype(F8)

        # K side
        kq = pair_major(np.asarray(inputs["k_quant"])[:, hsl])      # [U, D, SQ] int
        kcode = kq.astype(np.float32).astype(F8)
        ksc = pair_major(np.asarray(inputs["k_scale"], np.float32)[:, hsl])  # [U, 128, G]
        ksc64 = np.ascontiguousarray(
            (ksc * QS_SC).transpose(1, 0, 2)).astype(ml_dtypes.bfloat16)     # [128, U, G]
        kmn_ = pair_major(np.asarray(inputs["k_mn"], np.float32)[:, hsl])
        kmn = np.ascontiguousarray(kmn_.transpose(1, 0, 2)).astype(ml_dtypes.bfloat16)
        kfr = pair_major(np.asarray(inputs["k_full"], np.float32)[:, hsl])   # [U, SF, D]
        kfp = np.zeros((128, U, 128), np.float32)
        kfp[:, :, 0:SF] = kfr.transpose(2, 0, 1)
        kfp = kfp.astype(ml_dtypes.bfloat16)
        kpr = pair_major(np.asarray(inputs["key_p"], np.float32)[:, hsl])    # [U, SQ, R]
        kp = np.ascontiguousarray(
            kpr.reshape(U, NCH, 128, RANK).transpose(2, 0, 1, 3)).astype(ml_dtypes.bfloat16)
        keyq = np.ascontiguousarray(
            pair_major(np.asarray(inputs["key_q"], np.float32)[:, hsl]
                       ).transpose(1, 0, 2)).astype(ml_dtypes.bfloat16)      # [128, U, R]

        # V side: vt mega tile [U, 128, NCH*VT_W] fp8
        vq_ = pair_major(np.asarray(inputs["v_quant"])[:, hsl])     # [U, SQ, D] int
        vmn_ = pair_major(np.asarray(inputs["v_mn"], np.float32)[:, hsl])    # [U, SQ, 2]
        vlq_ = pair_major(np.asarray(inputs["value_q"], np.float32)[:, hsl])  # [U, SQ, R]
        vt = np.zeros((U, NCH, 128, VT_W), np.float32)
        vt[:, :, :, 0:128] = vq_.reshape(U, NCH, 128, D)
        vt[:, :, :, 128:130] = vmn_.reshape(U, NCH, 128, FD) * MN_SC
        # col 130 = aw slot (device-filled); rest pad
        vtall = np.ascontiguousarray(
            vt.transpose(0, 2, 1, 3).reshape(U, 128, NCH * VT_W)).astype(F8)

        vqst = np.zeros((U, NCH, 128, AW_W), np.float32)  # cols 0-2 device, 3:7 vq
        vqst[:, :, :, 3:7] = vlq_.reshape(U, NCH, 128, RANK) * MN_SC
        vqst = np.ascontiguousarray(
            vqst.transpose(2, 0, 1, 3).reshape(128, U, NCH * AW_W)).astype(F8)

        vsc_ = pair_major(np.asarray(inputs["v_scale"], np.float32)[:, hsl])  # [U, SQ, 2]
        vsp = np.ascontiguousarray(
            (vsc_ * VS_SC).reshape(U, NCH, 128, FD).transpose(2, 0, 1, 3)
        ).astype(ml_dtypes.bfloat16)                                          # [128, U, NCH, 2]

        vp_ = pair_major(np.asarray(inputs["value_p"], np.float32)[:, hsl])   # [U, D, R]
        vpT = np.zeros((7, U, D), np.float32)
        vpT[3:7] = vp_.transpose(2, 0, 1) / (AW_SC * MN_SC)
        vpT = vpT.astype(ml_dtypes.bfloat16)
        vfr_ = pair_major(np.asarray(inputs["v_full"], np.float32)[:, hsl])   # [U, SF, D]
        vfr = np.zeros((SFP, U, D), np.float32)
        vfr[0:SF] = vfr_.transpose(1, 0, 2)
        vfr = vfr.astype(ml_dtypes.bfloat16)

        m = {
            "hidb": hidb, "cost": cost, "sint": sint,
            "wqT": wchunk(wq[sl].T),
            "wkT": wchunk(wk[sl].T),
            "wvT": wchunk(wv[sl].T),
            "woT": np.ascontiguousarray(
                wo[:, sl].T.reshape(4, 128, HID).transpose(1, 0, 2)
            ).astype(ml_dtypes.bfloat16),
            "kcode": kcode, "ksc64": ksc64, "kmn": kmn, "kfp": kfp,
            "kp": kp, "keyq": keyq,
            "vtall": vtall, "vqst": vqst, "vsp": vsp, "vpT": vpT, "vfr": vfr,
        }
        in_maps.append(m)
    return in_maps


def kernel(**inputs):
    if "nc" not in _CACHE:
        _CACHE["nc"] = _build()
    nc = _CACHE["nc"]
    in_maps = _host_prep(inputs)
    res = run_bass_kernel_spmd(nc, in_maps, list(range(NCORES)),
                               trace=bool(os.environ.get("K_TRACE")))
    kernel.last = res
    total = np.zeros((B, HID), np.float32)
    for r in res.results:
        total += r["out"]
    return total.reshape(B, QL, HID)
